# revision 1
# baseline (speedup 1.0000x reference)
"""GCN (2-layer) SpMM kernel for 8 TRN2 NeuronCores via Bass/Tile.

Strategy (1D row partitioning, per sharding hint):
  - Destination rows sharded across 8 cores (12500 rows/core, padded to 12544).
  - support1 = x @ W1 computed fully (all nodes) on every core locally
    (cheap: avoids an extra collective), stored as an fp16 row-major table in
    HBM, laid out in "padded row space" (node n -> row m(n)).
  - SpMM per layer: edges of a core (contiguous, adj_row sorted) are grouped
    by (window of 64 dest rows, source-chunk, source-parity) on the host and
    padded to 128-edge tiles.  Per tile:
      * dma_gather pulls 256B "node pair" elements (2 adjacent fp16 rows) from
        the table into SBUF (edges on partitions),
      * one fused DVE tensor_scalar builds the selection matrix
        S[p, j] = val[p] * (rowloc[p] == j)  (iota == rowloc) * val,
      * one PE matmul accumulates S^T @ G_half into the window's PSUM bank.
    Parity pre-split on the host lets each tile read a single 64-col half of
    the gathered pair, so a single S per tile suffices.
  - Bias is added via a K=1 matmul (ones^T @ b) opening each PSUM window.
  - Layer-1 window close: LeakyReLU (ACT Lrelu), PE transpose, h @ W2 ->
    support2 shard; AllGather (fp16) makes the full support2 table; layer 2
    repeats the SpMM and writes fp32 output rows.

Self-contained: hardcodes all shapes; only needs the staged runtime
(concourse) available on the machine, as provided in this container.
"""

import os
import numpy as np

# ---------------------------------------------------------------- config ---


class Cfg:
    def __init__(self, N, E, D=64, NC=8, W=64, SUPER=4, XBLK=512, chunk_cap=25088):
        self.N, self.E, self.D, self.NC, self.W, self.SUPER = N, E, D, NC, W, SUPER
        self.R = N // NC                      # real rows per core
        self.NW = -(-self.R // W)             # windows per core
        # pad windows so NW % SUPER == 0
        self.NW = -(-self.NW // SUPER) * SUPER
        self.NSG = self.NW // SUPER
        self.RP = self.NW * W                 # padded rows per core
        self.NPAD = self.RP * NC              # padded table rows
        assert self.NPAD % 2 == 0
        self.NPAIRS = self.NPAD // 2
        # chunks of pairs, each < 32768 so chunk-relative pair idx fits int16
        self.NK = -(-self.NPAIRS // chunk_cap) if self.NPAIRS > chunk_cap else 1
        self.CHUNK = -(-self.NPAIRS // self.NK)
        assert self.CHUNK <= 32767
        self.XBLK = XBLK                      # rows per phase-1 block
        assert self.NPAD % XBLK == 0 and XBLK % 128 == 0
        self.PAR = 2
        self.MG = 8                           # max tiles per gather instr (1024 idxs = 65 descs; >8 crashes NRT)

    def m_of_node(self, n):
        """node id -> padded table row"""
        return (n // self.R) * self.RP + (n % self.R)


FULL = Cfg(N=100000, E=3200000)
LAST_EXEC_NS = None


# ------------------------------------------------------------- host prep ---


def prep(cfg, adj_row, adj_col, adj_val):
    """Build the shared tile schedule + per-core edge streams.

    Returns (tiles[NSG,NK,PAR,SUPER], per_core list of dicts with
    idx [16, SLOTS] int16, rowloc [128, T] fp16, val [128, T] fp16).
    """
    N, NC, W, SUPER, NK, PAR = cfg.N, cfg.NC, cfg.W, cfg.SUPER, cfg.NK, cfg.PAR
    NSG, CHUNK, R = cfg.NSG, cfg.CHUNK, cfg.R
    NWIN = cfg.NW

    row = np.asarray(adj_row, dtype=np.int64)
    col = np.asarray(adj_col, dtype=np.int64)
    val = np.asarray(adj_val, dtype=np.float32)

    bounds = np.searchsorted(row, np.arange(NC + 1) * R)
    cores = []
    ngroups = NWIN * NK * PAR
    counts = np.zeros((NC, ngroups), dtype=np.int64)
    for c in range(NC):
        e0, e1 = bounds[c], bounds[c + 1]
        r = (row[e0:e1] - c * R).astype(np.int64)
        m = cfg.m_of_node(col[e0:e1])
        v = val[e0:e1]
        w = r // W
        rowloc = r % W
        pair = m >> 1
        par = (m & 1).astype(np.int64)
        k = pair // CHUNK
        pidx = pair - k * CHUNK
        # group key, ordered (sg, k, par, w) to match emission order:
        # global order must be: for sg: for k: for par: for w in sg
        sg = w // SUPER
        w4 = w % SUPER
        key = ((sg * NK + k) * PAR + par) * SUPER + w4
        order = np.argsort(key, kind="stable")
        cores.append(
            dict(key=key[order], pidx=pidx[order], rowloc=rowloc[order], v=v[order])
        )
        counts[c] = np.bincount(key, minlength=ngroups)

    # shared schedule: tiles per group = max over cores of ceil(count/128)
    gtiles = -(-counts.max(axis=0) // 128)  # [ngroups]
    tiles = gtiles.reshape(NSG, NK, PAR, SUPER)
    T = int(gtiles.sum())
    T = max(T, 1)

    per_core = []
    for c in range(NC):
        d = cores[c]
        idx_s = np.zeros(T * 128, dtype=np.int16)
        rl_s = np.zeros(T * 128, dtype=np.float32)
        vl_s = np.zeros(T * 128, dtype=np.float32)
        gstart = np.concatenate([[0], np.cumsum(counts[c])])
        tstart = np.concatenate([[0], np.cumsum(gtiles)])
        for g in range(ngroups):
            cnt = counts[c][g]
            if cnt == 0:
                continue
            s0, t0 = gstart[g], tstart[g] * 128
            idx_s[t0 : t0 + cnt] = d["pidx"][s0 : s0 + cnt]
            rl_s[t0 : t0 + cnt] = d["rowloc"][s0 : s0 + cnt]
            vl_s[t0 : t0 + cnt] = d["v"][s0 : s0 + cnt]
        per_core.append(
            dict(
                idx=np.tile(idx_s.reshape(-1, 16).T, (1, 1)),  # [16, T*8]
                rowloc=rl_s.reshape(T, 128).T.copy(),          # [128, T]
                val=vl_s.reshape(T, 128).T.copy(),             # [128, T]
            )
        )
    return tiles, per_core, T


# -------------------------------------------------------- numpy emulator ---


def emulate(cfg, tiles, per_core, xpad16, W1, b1, W2, b2):
    """Emulate the device program exactly (fp16 tables, fp32 psum)."""
    D, W, SUPER, NK, PAR, NSG = cfg.D, cfg.W, cfg.SUPER, cfg.NK, cfg.PAR, cfg.NSG
    CHUNK = cfg.CHUNK
    # phase 1: support1 table (all cores identical)
    sup1 = (xpad16.astype(np.float32) @ W1.astype(np.float16).astype(np.float32))
    sup1 = sup1.astype(np.float16)
    outs = []
    shards = []
    for c in range(cfg.NC):
        pc = per_core[c]
        idx = pc["idx"].T.reshape(-1)  # unwrap [16, S] -> stream
        idx = (
            pc["idx"].T.reshape(-1, 16).reshape(-1)
        )  # slot-major: [S,16] -> stream i = s*16+j
        rowloc = pc["rowloc"].T.reshape(-1)
        val = pc["val"].T.reshape(-1)
        shards.append(_emu_layer(cfg, tiles, idx, rowloc, val, sup1, b1, W2, layer=0))
    s2full = np.concatenate(shards, axis=0).astype(np.float16)
    for c in range(cfg.NC):
        pc = per_core[c]
        idx = pc["idx"].T.reshape(-1, 16).reshape(-1)
        rowloc = pc["rowloc"].T.reshape(-1)
        val = pc["val"].T.reshape(-1)
        out = _emu_layer(cfg, tiles, idx, rowloc, val, s2full, b2, None, layer=1)
        outs.append(out)
    return np.concatenate(outs, axis=0)  # [NPAD, D] f32/f16 mix


def _emu_layer(cfg, tiles, idx, rowloc, val, table, b, W2, layer):
    D, W, SUPER, NK, PAR, NSG, CHUNK = (
        cfg.D, cfg.W, cfg.SUPER, cfg.NK, cfg.PAR, cfg.NSG, cfg.CHUNK,
    )
    table_pairs = table.reshape(-1, 2 * D)  # [NPAIRS, 128]
    res = np.zeros((cfg.RP, D), dtype=np.float32)
    pos = 0
    for sg in range(NSG):
        psum = np.zeros((SUPER, W, D), dtype=np.float32)
        psum += b.astype(np.float16).astype(np.float32)[None, None, :]
        for k in range(NK):
            for par in range(PAR):
                for w4 in range(SUPER):
                    for t in range(tiles[sg, k, par, w4]):
                        sl = slice(pos * 128, pos * 128 + 128)
                        g = table_pairs[k * CHUNK + idx[sl].astype(np.int64)]
                        gh = g[:, par * D : par * D + D].astype(np.float32)
                        S = np.zeros((128, W), dtype=np.float32)
                        S[np.arange(128), rowloc[sl].astype(np.int64)] = val[
                            sl
                        ].astype(np.float32)
                        psum[w4] += S.T @ gh
                        pos += 1
        for w4 in range(SUPER):
            w = sg * SUPER + w4
            if layer == 0:
                h = np.where(psum[w4] >= 0, psum[w4], 0.2 * psum[w4]).astype(
                    np.float16
                )
                s2 = (h.astype(np.float32) @ W2.astype(np.float16).astype(np.float32))
                res[w * W : (w + 1) * W] = s2
            else:
                res[w * W : (w + 1) * W] = psum[w4]
    return res.astype(np.float16) if layer == 0 else res


# --------------------------------------------------------- device program ---


def build_program(cfg, tiles):
    import concourse.bass as bass
    import concourse.bacc as bacc
    from concourse import mybir
    from concourse.tile import TileContext

    f16, f32, i16 = mybir.dt.float16, mybir.dt.float32, mybir.dt.int16
    D, W, SUPER, NK, PAR, NSG = cfg.D, cfg.W, cfg.SUPER, cfg.NK, cfg.PAR, cfg.NSG
    CHUNK, NPAD, RP, XBLK = cfg.CHUNK, cfg.NPAD, cfg.RP, cfg.XBLK
    MG = cfg.MG
    T = max(int(tiles.sum()), 1)
    SLOTS = T * 8

    nc = bacc.Bacc(num_devices=cfg.NC, num_swdge_queues=4,
                   dynamic_dma_scratch_size=65536)

    x16 = nc.declare_dram_parameter("x16", [NPAD, D], f16, isOutput=False)
    w1p = nc.declare_dram_parameter("w1", [D, D], f16, isOutput=False)
    w2p = nc.declare_dram_parameter("w2", [D, D], f16, isOutput=False)
    b1p = nc.declare_dram_parameter("b1", [1, D], f16, isOutput=False)
    b2p = nc.declare_dram_parameter("b2", [1, D], f16, isOutput=False)
    idxp = nc.declare_dram_parameter("idx", [16, SLOTS], i16, isOutput=False)
    rlp = nc.declare_dram_parameter("rowloc", [128, T], f32, isOutput=False)
    vlp = nc.declare_dram_parameter("val", [128, T], f32, isOutput=False)
    outp = nc.declare_dram_parameter("out", [RP, D], f32, isOutput=True)

    sup1 = nc.dram_tensor("sup1", [NPAD, D], f16)
    s2sh = nc.dram_tensor("s2sh", [RP, D], f16)
    s2full = nc.dram_tensor("s2full", [NPAD, D], f16, addr_space="Shared")

    eq = mybir.AluOpType.is_equal
    mult = mybir.AluOpType.mult

    with TileContext(nc) as tc:
        with (
            tc.tile_pool(name="const", bufs=1) as cp,
            tc.tile_pool(name="meta", bufs=1) as mp,
        ):
            w1s = cp.tile([D, D], f16, tag="w1")
            nc.sync.dma_start(out=w1s[:], in_=w1p[:])
            w2s = cp.tile([D, D], f16, tag="w2")
            nc.sync.dma_start(out=w2s[:], in_=w2p[:])
            b1s = cp.tile([1, D], f16, tag="b1")
            nc.sync.dma_start(out=b1s[:], in_=b1p[:])
            b2s = cp.tile([1, D], f16, tag="b2")
            nc.sync.dma_start(out=b2s[:], in_=b2p[:])
            ones = cp.tile([1, D], f16, tag="ones")
            nc.vector.memset(ones[:], 1.0)
            iota = cp.tile([128, W], f16, tag="iota")
            nc.gpsimd.iota(
                iota[:], [[1, W]], channel_multiplier=0,
                allow_small_or_imprecise_dtypes=True,
            )
            iotap = cp.tile([D, 1], f32, tag="iotap")
            nc.gpsimd.iota(
                iotap[:], [[1, 1]], channel_multiplier=1,
                allow_small_or_imprecise_dtypes=True,
            )
            ident = cp.tile([D, D], f16, tag="ident")
            nc.vector.tensor_scalar(
                out=ident[:], in0=iota[0:D, 0:D], scalar1=iotap[:], scalar2=None,
                op0=eq,
            )
            rls = mp.tile([128, T], f32, tag="rl")
            nc.sync.dma_start(out=rls[:], in_=rlp[:])
            vls = mp.tile([128, T], f32, tag="vl")
            nc.sync.dma_start(out=vls[:], in_=vlp[:])

            # ---------------- phase 1: support1 = x @ W1 (full table) -----
            with (
                tc.tile_pool(name="ph1", bufs=3) as p1,
                tc.tile_pool(name="ph1ps", bufs=4, space="PSUM") as pp1,
            ):
                for b in range(NPAD // XBLK):
                    xT = p1.tile([D, XBLK], f16, tag="xT")
                    nc.sync.dma_start_transpose(
                        out=xT[:], in_=x16[b * XBLK : (b + 1) * XBLK, :]
                    )
                    st = p1.tile([128, XBLK // 128, D], f16, tag="st")
                    for t in range(XBLK // 128):
                        ps = pp1.tile([128, D], f32, tag="ps")
                        nc.tensor.matmul(
                            ps[:], lhsT=xT[:, t * 128 : (t + 1) * 128], rhs=w1s[:],
                            start=True, stop=True,
                        )
                        nc.scalar.activation(
                            out=st[:, t, :], in_=ps[:],
                            func=mybir.ActivationFunctionType.Copy,
                        )
                    nc.sync.dma_start(
                        out=sup1[b * XBLK : (b + 1) * XBLK, :].rearrange(
                            "(t p) d -> p t d", p=128
                        ),
                        in_=st[:],
                    )

            # ---------------- SpMM layers --------------------------------
            with (
                tc.tile_pool(name="gp", bufs=3) as gp,
                tc.tile_pool(name="ip", bufs=3) as ip,
                tc.tile_pool(name="sp", bufs=3) as sp,
                tc.tile_pool(name="hp", bufs=3) as hp,
                tc.tile_pool(name="op", bufs=3) as op,
                tc.tile_pool(name="accp", bufs=5, space="PSUM") as accp,
                tc.tile_pool(name="ptp", bufs=1, space="PSUM") as ptp,
                tc.tile_pool(name="ps2p", bufs=1, space="PSUM") as ps2p,
            ):

                def spmm_layer(layer, table, bias_s):
                    pos = 0
                    gq = [0]
                    for sg in range(NSG):
                        left = [
                            int(tiles[sg, :, :, w4].sum()) for w4 in range(SUPER)
                        ]
                        psums = []
                        for w4 in range(SUPER):
                            ps = accp.tile([W, D], f32, tag="acc")
                            nc.tensor.matmul(
                                ps[:], lhsT=ones[:], rhs=bias_s[:],
                                start=True, stop=(left[w4] == 0),
                            )
                            psums.append(ps)
                        for k in range(NK):
                            for par in range(PAR):
                                cnt = int(tiles[sg, k, par].sum())
                                if cnt == 0:
                                    continue
                                # flat per-tile window assignment in order
                                wmap = []
                                for w4 in range(SUPER):
                                    wmap += [w4] * int(tiles[sg, k, par, w4])
                                for p0 in range(0, cnt, MG):
                                    pc = min(MG, cnt - p0)
                                    git = gp.tile([128, pc, 2 * D], f16, tag="G")
                                    idxt = ip.tile([128, pc * 8], i16, tag="idx")
                                    nc.sync.dma_start(
                                        out=idxt[:],
                                        in_=bass.AP(
                                            idxp, (pos + p0) * 8,
                                            [[0, 8], [SLOTS, 16], [1, pc * 8]],
                                        ),
                                    )
                                    nc.gpsimd.dma_gather(
                                        git[:],
                                        bass.AP(
                                            table, k * CHUNK * 2 * D,
                                            [[2 * D, CHUNK], [1, 2 * D]],
                                        ),
                                        idxt[:],
                                        num_idxs=pc * 128,
                                        num_idxs_reg=pc * 128,
                                        elem_size=2 * D,
                                        queue_num=gq[0] % 4,
                                    )
                                    gq[0] += 1
                                    Ss = sp.tile([128, pc, W], f16, tag="S")
                                    for tl in range(pc):
                                        w4 = wmap[p0 + tl]
                                        gpos = pos + p0 + tl
                                        nc.vector.tensor_scalar(
                                            out=Ss[:, tl, :], in0=iota[:],
                                            scalar1=rls[:, gpos : gpos + 1],
                                            scalar2=vls[:, gpos : gpos + 1],
                                            op0=eq, op1=mult,
                                        )
                                        left[w4] -= 1
                                        nc.tensor.matmul(
                                            psums[w4][:],
                                            lhsT=Ss[:, tl, :],
                                            rhs=git[:, tl, par * D : par * D + D],
                                            start=False,
                                            stop=(left[w4] == 0),
                                        )
                                pos += cnt
                        # window close
                        if layer == 0:
                            s2t = hp.tile([W, SUPER, D], f16, tag="s2t")
                        else:
                            s2t = op.tile([W, SUPER, D], f32, tag="outt")
                        for w4 in range(SUPER):
                            if layer == 0:
                                hpos = hp.tile([W, D], f16, tag="hpos")
                                nc.scalar.activation(
                                    out=hpos[:], in_=psums[w4][:],
                                    func=mybir.ActivationFunctionType.Relu,
                                )
                                hneg = hp.tile([W, D], f16, tag="hneg")
                                nc.vector.tensor_scalar(
                                    out=hneg[:], in0=psums[w4][:],
                                    scalar1=0.0, scalar2=0.2,
                                    op0=mybir.AluOpType.min, op1=mult,
                                )
                                hh = hp.tile([W, D], f16, tag="hh")
                                nc.vector.tensor_tensor(
                                    out=hh[:], in0=hpos[:], in1=hneg[:],
                                    op=mybir.AluOpType.add,
                                )
                                pt = ptp.tile([D, W], f16, tag="pt")
                                nc.tensor.transpose(pt[:], hh[:], ident[:])
                                hT = hp.tile([D, W], f16, tag="hT")
                                nc.scalar.activation(
                                    out=hT[:], in_=pt[:],
                                    func=mybir.ActivationFunctionType.Copy,
                                )
                                ps2 = ps2p.tile([W, D], f32, tag="ps2")
                                nc.tensor.matmul(
                                    ps2[:], lhsT=hT[:], rhs=w2s[:],
                                    start=True, stop=True,
                                )
                                nc.scalar.activation(
                                    out=s2t[:, w4, :], in_=ps2[:],
                                    func=mybir.ActivationFunctionType.Copy,
                                )
                            else:
                                nc.scalar.activation(
                                    out=s2t[:, w4, :], in_=psums[w4][:],
                                    func=mybir.ActivationFunctionType.Copy,
                                )
                        dst = s2sh if layer == 0 else outp
                        nc.sync.dma_start(
                            out=dst[
                                sg * SUPER * W : (sg + 1) * SUPER * W, :
                            ].rearrange("(t p) d -> p t d", p=W),
                            in_=s2t[:],
                        )

                spmm_layer(0, sup1, b1s)
                nc.gpsimd.collective_compute(
                    "AllGather",
                    mybir.AluOpType.bypass,
                    replica_groups=[list(range(cfg.NC))],
                    ins=[s2sh[:]],
                    outs=[s2full[:]],
                )
                spmm_layer(1, s2full, b2s)

    nc.compile()
    return nc


# ----------------------------------------------------------------- kernel ---


def make_inputs(cfg, x, adj_row, adj_col, adj_val, W1, b1, W2, b2):
    tiles, per_core, T = prep(cfg, adj_row, adj_col, adj_val)
    x = np.asarray(x, dtype=np.float32)
    xpad = np.zeros((cfg.NPAD, cfg.D), dtype=np.float16)
    xpad[cfg.m_of_node(np.arange(cfg.N))] = x.astype(np.float16)
    common = dict(
        x16=xpad,
        w1=np.asarray(W1, np.float16),
        w2=np.asarray(W2, np.float16),
        b1=np.asarray(b1, np.float16).reshape(1, cfg.D),
        b2=np.asarray(b2, np.float16).reshape(1, cfg.D),
    )
    in_maps = []
    for c in range(cfg.NC):
        m = dict(common)
        m["idx"] = per_core[c]["idx"]
        m["rowloc"] = per_core[c]["rowloc"]
        m["val"] = per_core[c]["val"]
        in_maps.append(m)
    return tiles, in_maps, xpad


def kernel(x, adj_row, adj_col, adj_val, W1, b1, W2, b2, _cfg=None, _sim=False):
    cfg = _cfg or FULL
    tiles, in_maps, _ = make_inputs(
        cfg, x, adj_row, adj_col, adj_val, W1, b1, W2, b2
    )
    nc = build_program(cfg, tiles)
    if _sim:
        from concourse import bass_interp

        sim = bass_interp.MultiCoreSim(nc, cfg.NC)
        for c in range(cfg.NC):
            for k, v in in_maps[c].items():
                sim.cores[c].tensor(k)[:] = v
        sim.simulate()
        results = [{"out": np.array(sim.cores[c].tensor("out"))} for c in range(cfg.NC)]
    else:
        from concourse.bass_utils import run_bass_kernel_spmd

        trace = bool(int(os.environ.get("GCN_TRACE", "0")))
        res = run_bass_kernel_spmd(nc, in_maps, list(range(cfg.NC)), trace=trace)
        results = res.results
        global LAST_EXEC_NS
        LAST_EXEC_NS = res.exec_time_ns
        if trace:
            print(f"HW exec time: {res.exec_time_ns} ns")
    out = np.empty((cfg.N, cfg.D), dtype=np.float32)
    for c in range(cfg.NC):
        out[c * cfg.R : (c + 1) * cfg.R] = results[c]["out"][: cfg.R]
    return out



# revision 13
# speedup vs baseline: 3.6767x; 3.6767x over previous
"""GCN (2-layer) SpMM kernel for 8 TRN2 NeuronCores via Bass/Tile.

Strategy (1D row partitioning, per sharding hint):
  - Destination rows sharded across 8 cores (12500 rows/core, padded to 12544).
  - support1 = x @ W1 computed fully (all nodes) on every core locally,
    stored as an fp16 row-major table in HBM in "padded row space"
    (node n -> row m(n)).  x is supplied pre-transposed + quad-interleaved so
    phase 1 needs no DMA transpose and stores in 512B-contiguous runs.
  - SpMM per layer: edges of a core (contiguous, adj_row sorted) are grouped
    by (window of 128 dest rows, quad-pair parity, row parity) on the host and
    padded to 128-edge tiles.  Per gather batch (<=8 tiles):
      * dma_gather pulls 256B "node pair" elements (2 adjacent fp16 rows,
        at 512B quad stride) from the table into SBUF (edges on partitions),
      * two DVE tensor_tensor ops build all selection matrices of the batch
        at once: S[p, t, j] = val[p,t] * (iota[j] == rowloc[p,t]),
      * one PE matmul per tile accumulates S^T @ G_half into its window PSUM.
    Gather indices are preloaded into SBUF in bulk (no per-gather DMA);
    trailing padding slots carry idx=-1 so the Q7 descgen skips them.
  - Bias is added via a K=1 matmul (ones^T @ b) opening each PSUM window.
  - Layer-1 window close: LeakyReLU (ACT Lrelu), PE transpose, h @ W2 ->
    support2 shard; AllGather (fp16) makes the full support2 table; layer 2
    repeats the SpMM and writes fp32 output rows.

Self-contained: hardcodes all shapes; only needs the staged runtime
(concourse) available on the machine, as provided in this container.
"""

import os
import numpy as np

# ---------------------------------------------------------------- config ---


class Cfg:
    def __init__(self, N, E, D=64, NC=8, W=128, SUPER=2, MG=8, IC=8192, QT=8):
        self.N, self.E, self.D, self.NC, self.W, self.SUPER = N, E, D, NC, W, SUPER
        self.R = N // NC                      # real rows per core
        self.NW = -(-self.R // W)             # windows per core
        self.NW = -(-self.NW // SUPER) * SUPER
        self.NSG = self.NW // SUPER
        self.RP = self.NW * W                 # padded rows per core
        self.NPAD = self.RP * NC              # padded table rows
        assert self.NPAD % 512 == 0
        self.NQUAD = self.NPAD // 4           # gather idx = quad index
        assert self.NQUAD <= 32767            # int16 idx
        self.QP = 2                           # pair-within-quad
        self.PAR = 2                          # row-within-pair
        self.MG = MG                          # max tiles per gather instr
        self.IC = IC                          # idx SBUF buffer columns
        self.QT = QT                          # phase-1 512-row blocks per batch

    def m_of_node(self, n):
        """node id -> padded table row"""
        return (n // self.R) * self.RP + (n % self.R)


FULL = Cfg(N=100000, E=3200000)
LAST_EXEC_NS = None


# ------------------------------------------------------------- host prep ---


def schedule(cfg, adj_row, adj_col, adj_val):
    """Group each core's edges by (supergroup, quad-parity, row-parity,
    window) and pad each group to whole 128-edge tiles with a schedule shared
    across cores (max tile count per group)."""
    N, NC, W, SUPER = cfg.N, cfg.NC, cfg.W, cfg.SUPER
    QP, PAR, NSG, R = cfg.QP, cfg.PAR, cfg.NSG, cfg.R

    row = np.asarray(adj_row, dtype=np.int64)
    col = np.asarray(adj_col, dtype=np.int64)
    val = np.asarray(adj_val, dtype=np.float32)

    bounds = np.searchsorted(row, np.arange(NC + 1) * R)
    cores = []
    ngroups = NSG * QP * PAR * SUPER
    counts = np.zeros((NC, ngroups), dtype=np.int64)
    for c in range(NC):
        e0, e1 = bounds[c], bounds[c + 1]
        r = (row[e0:e1] - c * R).astype(np.int64)
        m = cfg.m_of_node(col[e0:e1])
        v = val[e0:e1]
        w = r // W
        rowloc = r % W
        quad = m >> 2
        qp = (m >> 1) & 1
        par = m & 1
        sg = w // SUPER
        w4 = w % SUPER
        # emission order: for sg: for qp: for (par, w4) tiles
        key = ((sg * QP + qp) * PAR + par) * SUPER + w4
        order = np.argsort(key, kind="stable")
        cores.append(
            dict(key=key[order], qidx=quad[order], rowloc=rowloc[order], v=v[order])
        )
        counts[c] = np.bincount(key, minlength=ngroups)

    gtiles = -(-counts.max(axis=0) // 128)  # [ngroups]
    tiles = gtiles.reshape(NSG, QP, PAR, SUPER)
    T = max(int(gtiles.sum()), 1)
    return tiles, counts, cores, T


def plan_batches(cfg, tiles):
    """Gather-batch plan, shared by host prep and device program.

    Returns (batches, runs).  Each batch: dict(sg, qp, t0, pc, tmap) where
    tmap[tl] = (par, w4) for each tile in the batch.  Each run: (rt0, rt1)
    tile range whose idx columns are bulk-loaded into one SBUF buffer.
    """
    NSG, QP, PAR, SUPER, MG = cfg.NSG, cfg.QP, cfg.PAR, cfg.SUPER, cfg.MG
    batches = []
    t0 = 0
    for sg in range(NSG):
        for qp in range(QP):
            tmap = []
            for par in range(PAR):
                for w4 in range(SUPER):
                    tmap += [(par, w4)] * int(tiles[sg, qp, par, w4])
            cnt = len(tmap)
            for p0 in range(0, cnt, MG):
                pc = min(MG, cnt - p0)
                batches.append(
                    dict(sg=sg, qp=qp, t0=t0 + p0, pc=pc, tmap=tmap[p0 : p0 + pc])
                )
            t0 += cnt
    # pack batches into idx-buffer runs of <= IC columns (8 cols per tile)
    cap_tiles = cfg.IC // 8
    runs = []
    rt0 = 0
    cur = 0
    for b in batches:
        if b["t0"] + b["pc"] - rt0 > cap_tiles:
            runs.append((rt0, b["t0"]))
            rt0 = b["t0"]
        b["run"] = len(runs)
        cur = b["t0"] + b["pc"]
    runs.append((rt0, max(cur, rt0 + 1)))
    return batches, runs


def prep(cfg, adj_row, adj_col, adj_val):
    """Build the shared tile schedule + per-core edge streams.

    Returns (tiles, batches, runs, per_core list of dicts with
    idxrep [128, T*8] int16, rowloc [128, T] fp16, val [128, T] fp16, T).
    """
    tiles, counts, cores, T = schedule(cfg, adj_row, adj_col, adj_val)
    batches, runs = plan_batches(cfg, tiles)
    gtiles = tiles.reshape(-1)
    ngroups = gtiles.shape[0]
    tstart = np.concatenate([[0], np.cumsum(gtiles)])

    streams = []
    reals = []
    for c in range(cfg.NC):
        d = cores[c]
        idx_s = np.zeros(T * 128, dtype=np.int16)
        rl_s = np.zeros(T * 128, dtype=np.float16)
        vl_s = np.zeros(T * 128, dtype=np.float16)
        real = np.zeros(T * 128, dtype=bool)
        gstart = np.concatenate([[0], np.cumsum(counts[c])])
        for g in range(ngroups):
            cnt = int(counts[c][g])
            if cnt == 0:
                continue
            s0, t0 = gstart[g], tstart[g] * 128
            idx_s[t0 : t0 + cnt] = d["qidx"][s0 : s0 + cnt]
            rl_s[t0 : t0 + cnt] = d["rowloc"][s0 : s0 + cnt]
            vl_s[t0 : t0 + cnt] = d["v"][s0 : s0 + cnt]
            real[t0 : t0 + cnt] = True
        streams.append((idx_s, rl_s, vl_s))
        reals.append(real)

    # Padding slots keep idx=0 (a harmless real row; S has val=0 there).
    # Trailing -1 idx trimming would save ~5% of Q7 descgen but leaves
    # skipped SBUF slots stale, which the matmul still reads (0*NaN risk);
    # not worth the hazard.
    for b in batches:
        b["valid"] = b["pc"] * 128
    del reals

    per_core = []
    for c in range(cfg.NC):
        idx_s, rl_s, vl_s = streams[c]
        per_core.append(
            dict(
                idxrep=np.broadcast_to(
                    idx_s.reshape(-1, 16).T, (8, 16, T * 8)
                ).reshape(128, T * 8)
                .copy(),                                   # [128, T*8]
                rowloc=rl_s.reshape(T, 128).T.copy(),      # [128, T] fp16
                val=vl_s.reshape(T, 128).T.copy(),         # [128, T] fp16
            )
        )
    return tiles, batches, runs, per_core, T


# --------------------------------------------------------- device program ---


def build_program(cfg, tiles, batches, runs, T, lrelu_native=True):
    import concourse.bass as bass
    import concourse.bacc as bacc
    from concourse import mybir
    from concourse.tile import TileContext

    f16, f32, i16 = mybir.dt.float16, mybir.dt.float32, mybir.dt.int16
    D, W, SUPER, QP, PAR, NSG = cfg.D, cfg.W, cfg.SUPER, cfg.QP, cfg.PAR, cfg.NSG
    NPAD, RP, NQUAD, QT = cfg.NPAD, cfg.RP, cfg.NQUAD, cfg.QT
    SLOTS = T * 8
    NQB = NPAD // 512  # phase-1 quad blocks

    nc = bacc.Bacc(num_devices=cfg.NC, num_swdge_queues=4,
                   dynamic_dma_scratch_size=65536)

    xtq = nc.declare_dram_parameter("xtq", [D, NPAD], f16, isOutput=False)
    w1p = nc.declare_dram_parameter("w1", [D, D], f16, isOutput=False)
    w2p = nc.declare_dram_parameter("w2", [D, D], f16, isOutput=False)
    b1p = nc.declare_dram_parameter("b1", [1, D], f16, isOutput=False)
    b2p = nc.declare_dram_parameter("b2", [1, D], f16, isOutput=False)
    idxp = nc.declare_dram_parameter("idxrep", [128, SLOTS], i16, isOutput=False)
    rlp = nc.declare_dram_parameter("rowloc", [128, T], f16, isOutput=False)
    vlp = nc.declare_dram_parameter("val", [128, T], f16, isOutput=False)
    outp = nc.declare_dram_parameter("out", [RP, D], f32, isOutput=True)

    sup1 = nc.dram_tensor("sup1", [NPAD, D], f16)
    s2sh = nc.dram_tensor("s2sh", [RP, D], f16)
    s2full = nc.dram_tensor("s2full", [NPAD, D], f16, addr_space="Shared")

    eq = mybir.AluOpType.is_equal
    mult = mybir.AluOpType.mult

    with TileContext(nc) as tc:
        with (
            tc.tile_pool(name="const", bufs=1) as cp,
            tc.tile_pool(name="meta", bufs=1) as mp,
        ):
            w1s = cp.tile([D, D], f16, tag="w1")
            nc.sync.dma_start(out=w1s[:], in_=w1p[:])
            w2s = cp.tile([D, D], f16, tag="w2")
            nc.sync.dma_start(out=w2s[:], in_=w2p[:])
            b1s = cp.tile([1, D], f16, tag="b1")
            nc.sync.dma_start(out=b1s[:], in_=b1p[:])
            b2s = cp.tile([1, D], f16, tag="b2")
            nc.sync.dma_start(out=b2s[:], in_=b2p[:])
            ones = cp.tile([1, W], f16, tag="ones")
            nc.vector.memset(ones[:], 1.0)
            iota = cp.tile([128, W], f16, tag="iota")
            nc.gpsimd.iota(
                iota[:], [[1, W]], channel_multiplier=0,
                allow_small_or_imprecise_dtypes=True,
            )
            iotap = cp.tile([128, 1], f32, tag="iotap")
            nc.gpsimd.iota(
                iotap[:], [[1, 1]], channel_multiplier=1,
                allow_small_or_imprecise_dtypes=True,
            )
            ident = cp.tile([128, 128], f16, tag="ident")
            nc.vector.tensor_scalar(
                out=ident[:], in0=iota[:, 0:128], scalar1=iotap[:], scalar2=None,
                op0=eq,
            )
            rls = mp.tile([128, T], f16, tag="rl")
            nc.sync.dma_start(out=rls[:], in_=rlp[:])
            vls = mp.tile([128, T], f16, tag="vl")
            nc.sync.dma_start(out=vls[:], in_=vlp[:])

            # ---------------- phase 1: support1 = x @ W1 (full table) -----
            # x arrives transposed + quad-interleaved: column b*512 + r*128 + p
            # holds x row for table row (b*128 + p)*4 + r, so each psum
            # evacuates into 512B-contiguous quad rows.
            with (
                tc.tile_pool(name="ph1x", bufs=2) as xp,
                tc.tile_pool(name="ph1s", bufs=2) as stp,
                tc.tile_pool(name="ph1ps", bufs=4, space="PSUM") as pp1,
            ):
                for qb in range(0, NQB, QT):
                    nb = min(QT, NQB - qb)
                    xt = xp.tile([D, 512 * nb], f16, tag="xt")
                    nc.sync.dma_start(
                        out=xt[:], in_=xtq[:, qb * 512 : (qb + nb) * 512]
                    )
                    st = stp.tile([128, nb, 256], f16, tag="st")
                    for jj in range(nb):
                        for r4 in range(4):
                            ps = pp1.tile([128, D], f32, tag="ps")
                            nc.tensor.matmul(
                                ps[:],
                                lhsT=xt[:, jj * 512 + r4 * 128 : jj * 512 + (r4 + 1) * 128],
                                rhs=w1s[:],
                                start=True, stop=True,
                            )
                            nc.scalar.activation(
                                out=st[:, jj, r4 * D : (r4 + 1) * D], in_=ps[:],
                                func=mybir.ActivationFunctionType.Copy,
                            )
                    nc.sync.dma_start(
                        out=sup1[qb * 512 : (qb + nb) * 512, :].rearrange(
                            "(b p x) d -> p b (x d)", p=128, x=4
                        ),
                        in_=st[:],
                    )

            # ---------------- SpMM layers --------------------------------
            with (
                tc.tile_pool(name="ib", bufs=2) as ibp,
                tc.tile_pool(name="gp", bufs=3) as gp,
                tc.tile_pool(name="sp", bufs=3) as sp,
                tc.tile_pool(name="hp", bufs=3) as hp,
                tc.tile_pool(name="op", bufs=3) as op,
                tc.tile_pool(name="accp", bufs=5, space="PSUM") as accp,
                tc.tile_pool(name="ptp", bufs=1, space="PSUM") as ptp,
                tc.tile_pool(name="ps2p", bufs=1, space="PSUM") as ps2p,
            ):

                def spmm_layer(layer, table, bias_s):
                    gq = [0]
                    cur_run = [-1, None]  # run id, tile handle

                    def get_idx(b):
                        rid = b["run"]
                        if cur_run[0] != rid:
                            rt0, rt1 = runs[rid]
                            ncols = (rt1 - rt0) * 8
                            ibt = ibp.tile([128, ncols], i16, tag="ib")
                            nc.sync.dma_start(
                                out=ibt[:], in_=idxp[:, rt0 * 8 : rt1 * 8]
                            )
                            cur_run[0], cur_run[1] = rid, ibt
                        rt0 = runs[rid][0]
                        off = (b["t0"] - rt0) * 8
                        return cur_run[1][:, off : off + b["pc"] * 8]

                    bi = 0
                    for sg in range(NSG):
                        left = [
                            int(tiles[sg, :, :, w4].sum()) for w4 in range(SUPER)
                        ]
                        psums = []
                        for w4 in range(SUPER):
                            ps = accp.tile([W, D], f32, tag="acc")
                            nc.tensor.matmul(
                                ps[:], lhsT=ones[:], rhs=bias_s[:],
                                start=True, stop=(left[w4] == 0),
                            )
                            psums.append(ps)
                        while bi < len(batches) and batches[bi]["sg"] == sg:
                            b = batches[bi]
                            pc, qp, t0 = b["pc"], b["qp"], b["t0"]
                            idx_ap = get_idx(b)
                            git = gp.tile([128, pc, 2 * D], f16, tag="G")
                            nc.gpsimd.dma_gather(
                                git[:],
                                bass.AP(
                                    table, qp * 2 * D,
                                    [[4 * D, NQUAD], [1, 2 * D]],
                                ),
                                idx_ap,
                                num_idxs=pc * 128,
                                num_idxs_reg=b["valid"],
                                elem_size=2 * D,
                                elem_step=4 * D,
                                queue_num=gq[0] % 4,
                            )
                            gq[0] += 1
                            seq = sp.tile([128, pc, W], f16, tag="Seq")
                            nc.vector.tensor_tensor(
                                out=seq[:],
                                in0=iota[:].unsqueeze(1).broadcast_to([128, pc, W]),
                                in1=rls[:, t0 : t0 + pc]
                                .unsqueeze(2)
                                .broadcast_to([128, pc, W]),
                                op=eq,
                            )
                            Ss = sp.tile([128, pc, W], f16, tag="S")
                            nc.vector.tensor_tensor(
                                out=Ss[:],
                                in0=seq[:],
                                in1=vls[:, t0 : t0 + pc]
                                .unsqueeze(2)
                                .broadcast_to([128, pc, W]),
                                op=mult,
                            )
                            for tl in range(pc):
                                par, w4 = b["tmap"][tl]
                                left[w4] -= 1
                                nc.tensor.matmul(
                                    psums[w4][:],
                                    lhsT=Ss[:, tl, :],
                                    rhs=git[:, tl, par * D : par * D + D],
                                    start=False,
                                    stop=(left[w4] == 0),
                                )
                            bi += 1
                        # window close
                        if layer == 0:
                            s2t = hp.tile([W, SUPER, D], f16, tag="s2t")
                        else:
                            s2t = op.tile([W, SUPER, D], f32, tag="outt")
                        for w4 in range(SUPER):
                            if layer == 0:
                                hh = hp.tile([W, D], f16, tag="hh")
                                if lrelu_native:
                                    nc.scalar.activation(
                                        out=hh[:], in_=psums[w4][:],
                                        func=mybir.ActivationFunctionType.Lrelu,
                                        alpha=0.2,
                                    )
                                else:
                                    hpos = hp.tile([W, D], f16, tag="hpos")
                                    nc.scalar.activation(
                                        out=hpos[:], in_=psums[w4][:],
                                        func=mybir.ActivationFunctionType.Relu,
                                    )
                                    hneg = hp.tile([W, D], f16, tag="hneg")
                                    nc.vector.tensor_scalar(
                                        out=hneg[:], in0=psums[w4][:],
                                        scalar1=0.0, scalar2=0.2,
                                        op0=mybir.AluOpType.min, op1=mult,
                                    )
                                    nc.vector.tensor_tensor(
                                        out=hh[:], in0=hpos[:], in1=hneg[:],
                                        op=mybir.AluOpType.add,
                                    )
                                pt = ptp.tile([D, W], f16, tag="pt")
                                nc.tensor.transpose(pt[:], hh[:], ident[:])
                                hT = hp.tile([D, W], f16, tag="hT")
                                nc.scalar.activation(
                                    out=hT[:], in_=pt[:],
                                    func=mybir.ActivationFunctionType.Copy,
                                )
                                ps2 = ps2p.tile([W, D], f32, tag="ps2")
                                nc.tensor.matmul(
                                    ps2[:], lhsT=hT[:], rhs=w2s[:],
                                    start=True, stop=True,
                                )
                                nc.scalar.activation(
                                    out=s2t[:, w4, :], in_=ps2[:],
                                    func=mybir.ActivationFunctionType.Copy,
                                )
                            else:
                                nc.scalar.activation(
                                    out=s2t[:, w4, :], in_=psums[w4][:],
                                    func=mybir.ActivationFunctionType.Copy,
                                )
                        dst = s2sh if layer == 0 else outp
                        nc.sync.dma_start(
                            out=dst[
                                sg * SUPER * W : (sg + 1) * SUPER * W, :
                            ].rearrange("(t p) d -> p t d", p=W),
                            in_=s2t[:],
                        )

                spmm_layer(0, sup1, b1s)
                nc.gpsimd.collective_compute(
                    "AllGather",
                    mybir.AluOpType.bypass,
                    replica_groups=[list(range(cfg.NC))],
                    ins=[s2sh[:]],
                    outs=[s2full[:]],
                )
                spmm_layer(1, s2full, b2s)

    nc.compile()
    return nc


# ----------------------------------------------------------------- kernel ---


def make_inputs(cfg, x, adj_row, adj_col, adj_val, W1, b1, W2, b2):
    tiles, batches, runs, per_core, T = prep(cfg, adj_row, adj_col, adj_val)
    x = np.asarray(x, dtype=np.float32)
    xpad = np.zeros((cfg.NPAD, cfg.D), dtype=np.float16)
    xpad[cfg.m_of_node(np.arange(cfg.N))] = x.astype(np.float16)
    # transpose + quad interleave: col b*512 + r*128 + p <- row (b*128+p)*4+r
    xq = xpad.reshape(-1, 128, 4, cfg.D)           # [b, p, r, d]
    xq = xq.transpose(0, 2, 1, 3).reshape(cfg.NPAD, cfg.D)  # [b*512+r*128+p, d]
    xtq = np.ascontiguousarray(xq.T)               # [D, NPAD]
    common = dict(
        xtq=xtq,
        w1=np.asarray(W1, np.float16),
        w2=np.asarray(W2, np.float16),
        b1=np.asarray(b1, np.float16).reshape(1, cfg.D),
        b2=np.asarray(b2, np.float16).reshape(1, cfg.D),
    )
    in_maps = []
    for c in range(cfg.NC):
        m = dict(common)
        m["idxrep"] = per_core[c]["idxrep"]
        m["rowloc"] = per_core[c]["rowloc"]
        m["val"] = per_core[c]["val"]
        in_maps.append(m)
    return tiles, batches, runs, in_maps, T


def kernel(x, adj_row, adj_col, adj_val, W1, b1, W2, b2, _cfg=None, _sim=False):
    cfg = _cfg or FULL
    tiles, batches, runs, in_maps, T = make_inputs(
        cfg, x, adj_row, adj_col, adj_val, W1, b1, W2, b2
    )
    # Native ACT Lrelu mis-evaluates on HW (rel err 0.2 observed); keep the
    # Relu + min/mult + add composition on both paths.
    nc = build_program(cfg, tiles, batches, runs, T, lrelu_native=False)
    if _sim:
        from concourse import bass_interp

        sim = bass_interp.MultiCoreSim(nc, cfg.NC)
        for c in range(cfg.NC):
            for k, v in in_maps[c].items():
                sim.cores[c].tensor(k)[:] = v
        sim.simulate()
        results = [{"out": np.array(sim.cores[c].tensor("out"))} for c in range(cfg.NC)]
    else:
        from concourse.bass_utils import run_bass_kernel_spmd

        trace = bool(int(os.environ.get("GCN_TRACE", "0")))
        res = run_bass_kernel_spmd(nc, in_maps, list(range(cfg.NC)), trace=trace)
        results = res.results
        global LAST_EXEC_NS
        LAST_EXEC_NS = res.exec_time_ns
        if trace:
            print(f"HW exec time: {res.exec_time_ns} ns")
    out = np.empty((cfg.N, cfg.D), dtype=np.float32)
    for c in range(cfg.NC):
        out[c * cfg.R : (c + 1) * cfg.R] = results[c]["out"][: cfg.R]
    return out
